# revision 1
# baseline (speedup 1.0000x reference)
"""AttentionBlock (GroupNorm -> qkv -> single-head attention L=4096 -> proj -> residual)
on 8 Trainium2 NeuronCores, data-parallel over the batch (B=8, one batch element per core).

Self-contained: hardcodes shapes B=8, C=512, L=4096, GROUPS=8.
"""
import sys
sys.path.insert(0, '/opt/trn_rl_repo')
import numpy as np
import concourse.bass as bass
import concourse.tile as tile
from concourse import mybir
from concourse.bass_utils import run_bass_kernel_spmd

B, C, L = 8, 512, 4096
G = 8                    # groups
GS = C // G              # 64 channels per group
CT = C // 128            # 4 channel partition-tiles
NOT = 3 * C // 128       # 12 qkv output row tiles
NCH = 512                # column chunk width
LC = L // NCH            # 8 l-chunks
KT = L // 128            # 32 k partition tiles
EPS = 1e-5
SCALE = 1.0 / float(np.sqrt(C))

f32 = mybir.dt.float32
f32r = mybir.dt.float32r
bf16 = mybir.dt.bfloat16
npbf16 = mybir.dt.np(bf16)

MAX_WAITS = 1
_split_ctr = [0]


def _split_multi_waits(nc):
    """walrus in this container rejects >1 sync wait per instruction.
    Hoist overflow waits onto same-engine NoOps inserted just before."""
    for f in nc.m.functions:
        for bb in f.blocks:
            new_insts = []
            for inst in bb.instructions:
                si = getattr(inst, 'sync_info', None)
                waits = list(si.on_wait) if si is not None and si.on_wait else []
                if len(waits) > MAX_WAITS:
                    overflow, keep = waits[:-MAX_WAITS], waits[-MAX_WAITS:]
                    for i in range(0, len(overflow), MAX_WAITS):
                        chunk = overflow[i:i + MAX_WAITS]
                        _split_ctr[0] += 1
                        noop = mybir.InstNoOp(
                            name=f"wait-split-{_split_ctr[0]}",
                            engine=inst.engine,
                            sync_info=mybir.SyncInfo(on_wait=chunk, on_update=[]),
                            bass_nofuse=True,
                        )
                        new_insts.append(noop)
                    inst.sync_info = mybir.SyncInfo(on_wait=keep, on_update=si.on_update)
                new_insts.append(inst)
            bb.instructions = new_insts


def build_nc(split=True):
    nc = bass.Bass("TRN2", num_devices=8)

    x_d = nc.dram_tensor("x", [C, L], f32, kind="ExternalInput")
    xh_d = nc.dram_tensor("xh", [C, L], bf16, kind="ExternalInput")
    gamma_d = nc.dram_tensor("gamma", [C], f32, kind="ExternalInput")
    beta_d = nc.dram_tensor("beta", [C], f32, kind="ExternalInput")
    wqkvT_d = nc.dram_tensor("wqkvT", [C, 3 * C], bf16, kind="ExternalInput")
    bqkv_d = nc.dram_tensor("bqkv", [3 * C], f32, kind="ExternalInput")
    woutT_d = nc.dram_tensor("woutT", [C, C], bf16, kind="ExternalInput")
    bout_d = nc.dram_tensor("bout", [C], f32, kind="ExternalInput")
    out_d = nc.dram_tensor("out", [C, L], f32, kind="ExternalOutput")

    # inline constants
    m_c2g = np.zeros((C, G), np.float32)
    for c in range(C):
        m_c2g[c, c // GS] = 1.0 / GS
    mask_c2g_d = nc.inline_tensor(m_c2g, "mask_c2g")
    m_g2c = np.zeros((G, C), np.float32)
    for c in range(C):
        m_g2c[c // GS, c] = 1.0
    mask_g2c_d = nc.inline_tensor(m_g2c, "mask_g2c")
    ident_d = nc.inline_tensor(np.eye(128, dtype=npbf16), "ident")
    ones128_d = nc.inline_tensor(np.ones((128, 128), npbf16), "ones128")
    ones128f_d = nc.inline_tensor(np.ones((128, 128), np.float32), "ones128f")

    with tile.TileContext(nc) as tc:
        with tc.tile_pool(name="singles", bufs=1) as singles:
            # ---- weight/bias/const loads (gpsimd queue: off the x-stream path) ----
            wqkvT = [singles.tile([128, 3 * C], bf16, tag=f"wq{t}", name=f"wq{t}") for t in range(CT)]
            for t in range(CT):
                nc.gpsimd.dma_start(out=wqkvT[t], in_=wqkvT_d[t * 128:(t + 1) * 128, :])
            woutT = [singles.tile([128, C], bf16, tag=f"wo{t}", name=f"wo{t}") for t in range(CT)]
            for t in range(CT):
                nc.gpsimd.dma_start(out=woutT[t], in_=woutT_d[t * 128:(t + 1) * 128, :])
            bqkv_sb = singles.tile([128, NOT], f32, tag="bqkv", name="bqkv")
            nc.gpsimd.dma_start(out=bqkv_sb, in_=bqkv_d[:].rearrange("(t p) -> p t", p=128))
            bout_sb = singles.tile([128, CT], f32, tag="bout", name="bout")
            nc.gpsimd.dma_start(out=bout_sb, in_=bout_d[:].rearrange("(t p) -> p t", p=128))
            gamma_sb = singles.tile([128, CT], f32, tag="gamma", name="gamma")
            nc.gpsimd.dma_start(out=gamma_sb, in_=gamma_d[:].rearrange("(t p) -> p t", p=128))
            beta_sb = singles.tile([128, CT], f32, tag="beta", name="beta")
            nc.gpsimd.dma_start(out=beta_sb, in_=beta_d[:].rearrange("(t p) -> p t", p=128))
            mask_c2g = [singles.tile([128, G], f32, tag=f"mc2g{t}", name=f"mc2g{t}") for t in range(CT)]
            for t in range(CT):
                nc.gpsimd.dma_start(out=mask_c2g[t], in_=mask_c2g_d[t * 128:(t + 1) * 128, :])
            mask_g2c = singles.tile([G, C], f32, tag="mg2c", name="mg2c")
            nc.gpsimd.dma_start(out=mask_g2c, in_=mask_g2c_d[:, :])
            ident = singles.tile([128, 128], bf16, tag="ident", name="ident")
            nc.gpsimd.dma_start(out=ident, in_=ident_d[:, :])
            ones128 = singles.tile([128, 128], bf16, tag="ones128", name="ones128")
            nc.gpsimd.dma_start(out=ones128, in_=ones128_d[:, :])
            ones128f = singles.tile([128, 128], f32r, tag="ones128f", name="ones128f")
            nc.gpsimd.dma_start(out=ones128f, in_=ones128f_d[:, :].bitcast(f32r))
            eps8 = singles.tile([G, 1], f32, tag="eps8", name="eps8")
            nc.vector.memset(eps8, EPS)

            # per-channel norm scale/offset (computed in stats phase)
            s_c = [singles.tile([128, 1], f32, tag=f"s_c{t}", name=f"s_c{t}") for t in range(CT)]
            t_c = [singles.tile([128, 1], f32, tag=f"t_c{t}", name=f"t_c{t}") for t in range(CT)]

            # q, k in [c, l] bf16; vT in [k(l), c] bf16
            q_sb = [singles.tile([128, L], bf16, tag=f"q{t}", name=f"q{t}") for t in range(CT)]
            k_sb = [singles.tile([128, L], bf16, tag=f"k{t}", name=f"k{t}") for t in range(CT)]
            vT = [singles.tile([128, C], bf16, tag=f"vT{kt}", name=f"vT{kt}") for kt in range(KT)]

            # ---- Phases A+B share SBUF-resident x; released before phase C ----
            with tc.tile_pool(name="xpool", bufs=1) as xpool:
                x_sb = [xpool.tile([128, L], bf16, tag=f"x{t}", name=f"x{t}") for t in range(CT)]

                # ---- Phase A: GroupNorm statistics (stream x once, keep it) ----
                with (
                    tc.tile_pool(name="stats", bufs=1) as stats,
                    tc.tile_pool(name="statps", bufs=1, space="PSUM") as statps,
                    tc.tile_pool(name="stmp", bufs=8) as stmp,
                ):
                    bn = [stats.tile([128, LC, 6], f32, tag=f"bn{t}", name=f"bn{t}") for t in range(CT)]
                    for t in range(CT):
                        for lc in range(LC):
                            xc = x_sb[t][:, lc * NCH:(lc + 1) * NCH]
                            eng = (nc.sync, nc.gpsimd, nc.scalar)[(t * LC + lc) % 3]
                            eng.dma_start(
                                out=xc, in_=xh_d[t * 128:(t + 1) * 128, lc * NCH:(lc + 1) * NCH])
                            nc.vector.bn_stats(out=bn[t][:, lc, :], in_=xc)
                    # per-channel mean/E[x^2] -> stats2[t] [128,2]
                    stats2 = [stats.tile([128, 2], f32, tag=f"st2{t}", name=f"st2{t}") for t in range(CT)]
                    for t in range(CT):
                        mv = stmp.tile([128, 2], f32, tag="mv", name="mv")
                        nc.vector.bn_aggr(out=mv, in_=bn[t])
                        sq = stmp.tile([128, 1], f32, tag="sq", name="sq")
                        nc.vector.tensor_mul(out=sq, in0=mv[:, 0:1], in1=mv[:, 0:1])
                        nc.vector.tensor_copy(out=stats2[t][:, 0:1], in_=mv[:, 0:1])
                        nc.vector.tensor_add(out=stats2[t][:, 1:2], in0=mv[:, 1:2], in1=sq)
                    # group aggregation: [8,2] = sum_t mask_c2g[t]^T @ stats2[t]
                    ps_g = statps.tile([G, 2], f32, tag="psg", name="psg")
                    for t in range(CT):
                        nc.tensor.matmul(ps_g, lhsT=mask_c2g[t], rhs=stats2[t],
                                         start=(t == 0), stop=(t == CT - 1))
                    gs = stmp.tile([G, 2], f32, tag="gs", name="gs")       # mean_g, E[x2]_g
                    nc.vector.tensor_copy(out=gs, in_=ps_g)
                    mg2 = stmp.tile([G, 1], f32, tag="mg2", name="mg2")
                    varg = stmp.tile([G, 1], f32, tag="varg", name="varg")
                    nc.vector.tensor_mul(out=mg2, in0=gs[:, 0:1], in1=gs[:, 0:1])
                    nc.vector.tensor_sub(out=varg, in0=gs[:, 1:2], in1=mg2)
                    # rstd_g = 1/sqrt(var+eps)
                    nc.scalar.activation(out=varg, in_=varg,
                                         func=mybir.ActivationFunctionType.Sqrt,
                                         bias=eps8, scale=1.0)
                    gstats = stmp.tile([G, 2], f32, tag="gstats", name="gstats")  # mean_g, rstd_g
                    nc.vector.tensor_copy(out=gstats[:, 0:1], in_=gs[:, 0:1])
                    nc.vector.reciprocal(out=gstats[:, 1:2], in_=varg)
                    # broadcast to channels; s_c = gamma*rstd, t_c = beta - mean*s_c
                    for t in range(CT):
                        ps_bc = statps.tile([128, 2], f32, tag="psbc", name="psbc")
                        nc.tensor.matmul(ps_bc, lhsT=mask_g2c[:, t * 128:(t + 1) * 128],
                                         rhs=gstats, start=True, stop=True)
                        bc = stmp.tile([128, 2], f32, tag="bc", name="bc")
                        nc.vector.tensor_copy(out=bc, in_=ps_bc)
                        nc.vector.tensor_mul(out=s_c[t], in0=gamma_sb[:, t:t + 1], in1=bc[:, 1:2])
                        tmp = stmp.tile([128, 1], f32, tag="tmp", name="tmp")
                        nc.vector.tensor_mul(out=tmp, in0=bc[:, 0:1], in1=s_c[t])
                        nc.vector.tensor_sub(out=t_c[t], in0=beta_sb[:, t:t + 1], in1=tmp)

                # ---- Phase B: qkv projection (+ V transpose), x already in SBUF ----
                with (
                    tc.tile_pool(name="xq", bufs=8) as xq,
                    tc.tile_pool(name="vtmp", bufs=3) as vtmp,
                    tc.tile_pool(name="qps", bufs=6, space="PSUM") as qps,
                    tc.tile_pool(name="tps", bufs=2, space="PSUM") as tps,
                ):
                    for lc in range(LC):
                        xn = []
                        for t in range(CT):
                            xn_t = xq.tile([128, NCH], bf16, tag="xn", name="xn")
                            nc.vector.tensor_scalar(
                                out=xn_t, in0=x_sb[t][:, lc * NCH:(lc + 1) * NCH],
                                scalar1=s_c[t], scalar2=t_c[t],
                                op0=mybir.AluOpType.mult,
                                op1=mybir.AluOpType.add)
                            xn.append(xn_t)
                        for ot in range(NOT):
                            ps = qps.tile([128, NCH], f32, tag="qps", name="qps")
                            for t in range(CT):
                                nc.tensor.matmul(ps, lhsT=wqkvT[t][:, ot * 128:(ot + 1) * 128],
                                                 rhs=xn[t], start=(t == 0), stop=(t == CT - 1))
                            if ot < CT:          # Q rows
                                dest = q_sb[ot][:, lc * NCH:(lc + 1) * NCH]
                                nc.scalar.add(out=dest, in_=ps, add=bqkv_sb[:, ot:ot + 1])
                            elif ot < 2 * CT:    # K rows
                                dest = k_sb[ot - CT][:, lc * NCH:(lc + 1) * NCH]
                                nc.scalar.add(out=dest, in_=ps, add=bqkv_sb[:, ot:ot + 1])
                            else:                # V rows -> transpose into vT
                                ct = ot - 2 * CT
                                vt_s = vtmp.tile([128, NCH], bf16, tag="vtmp", name="vtmp")
                                nc.scalar.add(out=vt_s, in_=ps, add=bqkv_sb[:, ot:ot + 1])
                                for j in range(NCH // 128):
                                    pt = tps.tile([128, 128], bf16, tag="tps", name="tps")
                                    nc.tensor.transpose(pt, vt_s[:, j * 128:(j + 1) * 128], ident)
                                    kt = lc * (NCH // 128) + j
                                    nc.vector.tensor_copy(
                                        out=vT[kt][:, ct * 128:(ct + 1) * 128], in_=pt)

            # ---- Phase C: attention + output projection + residual ----
            with (
                tc.tile_pool(name="exps", bufs=1) as exps,
                tc.tile_pool(name="sps", bufs=3, space="PSUM") as sps,
                tc.tile_pool(name="ops", bufs=1, space="PSUM") as ops,
                tc.tile_pool(name="dps", bufs=1, space="PSUM") as dps,
                tc.tile_pool(name="cwork", bufs=2) as cwork,
                tc.tile_pool(name="xres", bufs=4) as xres,
                tc.tile_pool(name="yout", bufs=4) as yout,
            ):
                for lc in range(LC):
                    ps_o = [ops.tile([128, NCH], f32, tag=f"o{ct}", name=f"o{ct}") for ct in range(CT)]
                    # den accumulated pre-broadcast: every row of ones128.T @ expS is sum_k
                    ps_den = dps.tile([128, NCH], f32, tag="den", name="den")
                    expS = []
                    # residual x (+ b_out) prefetched early, off the critical path
                    xb = []
                    for ot in range(CT):
                        xr = xres.tile([128, NCH], f32, tag="xr", name="xr")
                        nc.sync.dma_start(
                            out=xr, in_=x_d[ot * 128:(ot + 1) * 128, lc * NCH:(lc + 1) * NCH])
                        nc.vector.tensor_scalar(out=xr, in0=xr,
                                                scalar1=bout_sb[:, ot:ot + 1], scalar2=1.0,
                                                op0=mybir.AluOpType.add,
                                                op1=mybir.AluOpType.mult)
                        xb.append(xr)
                    for kt in range(KT):
                        ps_s = sps.tile([128, NCH], f32, tag="s", name="s")
                        for t in range(CT):
                            nc.tensor.matmul(
                                ps_s, lhsT=k_sb[t][:, kt * 128:(kt + 1) * 128],
                                rhs=q_sb[t][:, lc * NCH:(lc + 1) * NCH],
                                start=(t == 0), stop=(t == CT - 1))
                        es = exps.tile([128, NCH], bf16, tag=f"e{kt}", name=f"e{kt}")
                        nc.scalar.activation(out=es, in_=ps_s,
                                             func=mybir.ActivationFunctionType.Exp,
                                             scale=SCALE)
                        expS.append(es)
                        if kt % 4 == 3:
                            g = kt // 4
                            e0, e1, e2, e3 = expS[4 * g:4 * g + 4]
                            u0 = cwork.tile([128, NCH], f32, tag="u0", name="u0")
                            nc.vector.tensor_add(out=u0, in0=e0, in1=e1)
                            u1 = cwork.tile([128, NCH], f32, tag="u1", name="u1")
                            nc.vector.tensor_add(out=u1, in0=e2, in1=e3)
                            ug = cwork.tile([128, NCH], f32r, tag="ug", name="ug")
                            nc.vector.tensor_add(out=ug, in0=u0, in1=u1)
                            nc.tensor.matmul(ps_den, lhsT=ones128f, rhs=ug,
                                             start=(g == 0), stop=(g == KT // 4 - 1))
                        for ct in range(CT):
                            nc.tensor.matmul(
                                ps_o[ct], lhsT=vT[kt][:, ct * 128:(ct + 1) * 128],
                                rhs=es, start=(kt == 0), stop=(kt == KT - 1))
                    # 1/den commutes through the projection (per-column scaling):
                    # proj runs on unnormalized attn out; divide at the very end.
                    den_r = cwork.tile([128, NCH], f32, tag="den_r", name="den_r")
                    nc.vector.reciprocal(out=den_r, in_=ps_den)
                    ao = []
                    for ct in range(CT):
                        a = cwork.tile([128, NCH], bf16, tag=f"ao{ct}", name=f"ao{ct}")
                        nc.scalar.copy(out=a, in_=ps_o[ct])
                        ao.append(a)
                    for ot in range(CT):
                        ps_p = sps.tile([128, NCH], f32, tag="s", name="s")
                        for ct in range(CT):
                            nc.tensor.matmul(ps_p, lhsT=woutT[ct][:, ot * 128:(ot + 1) * 128],
                                             rhs=ao[ct], start=(ct == 0), stop=(ct == CT - 1))
                        y = yout.tile([128, NCH], f32, tag="y", name="y")
                        nc.vector.tensor_mul(out=y, in0=ps_p, in1=den_r)
                        nc.vector.tensor_add(out=y, in0=y, in1=xb[ot])
                        nc.sync.dma_start(
                            out=out_d[ot * 128:(ot + 1) * 128, lc * NCH:(lc + 1) * NCH], in_=y)

    if split:
        _split_multi_waits(nc)
    return nc


_NC_CACHE = [None]


def make_in_maps(x, gamma, beta, w_qkv, b_qkv, w_out, b_out):
    x = np.ascontiguousarray(np.asarray(x, dtype=np.float32))
    common = {
        "gamma": np.ascontiguousarray(np.asarray(gamma, np.float32)),
        "beta": np.ascontiguousarray(np.asarray(beta, np.float32)),
        "wqkvT": np.ascontiguousarray(np.asarray(w_qkv, np.float32).T.astype(npbf16)),
        "bqkv": np.ascontiguousarray(np.asarray(b_qkv, np.float32)),
        "woutT": np.ascontiguousarray(np.asarray(w_out, np.float32).T.astype(npbf16)),
        "bout": np.ascontiguousarray(np.asarray(b_out, np.float32)),
    }
    return [dict(common, x=np.ascontiguousarray(x[i]),
                 xh=np.ascontiguousarray(x[i].astype(npbf16))) for i in range(B)]


def kernel(x, gamma, beta, w_qkv, b_qkv, w_out, b_out):
    if _NC_CACHE[0] is None:
        _NC_CACHE[0] = build_nc()
    in_maps = make_in_maps(x, gamma, beta, w_qkv, b_qkv, w_out, b_out)
    res = run_bass_kernel_spmd(_NC_CACHE[0], in_maps, core_ids=list(range(B)))
    out = np.stack([res.results[i]["out"] for i in range(B)], axis=0)
    return out.astype(np.float32)



# revision 17
# speedup vs baseline: 1.6104x; 1.6104x over previous
"""AttentionBlock (GroupNorm -> qkv -> single-head attention L=4096 -> proj -> residual)
on 8 Trainium2 NeuronCores, data-parallel over the batch (B=8, one batch element per core).

v2: all four GEMMs in fp8e4 DoubleRow (2x PE throughput).
 - q/k/v/E/attn-out quantized to fp8 with static scales (w*8, ao/64, exp offset e^-3),
   all of which cancel exactly in the softmax ratio / final rescale.
 - V^T produced directly from the qkv matmul by swapping stationary/moving operands
   (no PE transposes, no transpose copies).
 - softmax denominator via 0.125-valued fp8 ones matmul on E pairs (no DVE add tree);
   the 0.125 pre-folds the ao*64 rescale: recip(den/8)*W8*(ao/64) = W*ao/den.
 - scalar engine runs exp only; bias/convert ops on gpsimd; residual+bias fused into
   one DVE scalar_tensor_tensor reading the f32 x kept resident in SBUF.

Self-contained: hardcodes shapes B=8, C=512, L=4096, GROUPS=8.
"""
import sys
sys.path.insert(0, '/opt/trn_rl_repo')
import numpy as np
import concourse.bass as bass
import concourse.tile as tile
from concourse import mybir
from concourse.bass_utils import run_bass_kernel_spmd

B, C, L = 8, 512, 4096
G = 8                    # groups
GS = C // G              # 64 channels per group
CT = C // 128            # 4 channel partition-tiles
NP = CT // 2             # 2 channel pair-tiles (DoubleRow contraction pairs)
NCH = 512                # column chunk width
LC = L // NCH            # 8 l-chunks
KT = L // 128            # 32 k partition tiles
KP = KT // 2             # 16 k pair-tiles
EPS = 1e-5
SCALE = 1.0 / float(np.sqrt(C))
C0 = 3.0                 # global exp offset: exp(s*SCALE - C0); cancels in softmax
WS = 8.0                 # weight quantization scale (w*8 in fp8)
AOS = 1.0 / 64.0         # attention-out quantization scale

f32 = mybir.dt.float32
f32r = mybir.dt.float32r
bf16 = mybir.dt.bfloat16
fp8 = mybir.dt.float8e4
npfp8 = mybir.dt.np(fp8)
DR = mybir.MatmulPerfMode.DoubleRow

MAX_WAITS = 1
_split_ctr = [0]


def _split_multi_waits(nc):
    """walrus in this container rejects >1 sync wait per instruction.
    Hoist overflow waits onto same-engine NoOps inserted just before."""
    for f in nc.m.functions:
        for bb in f.blocks:
            new_insts = []
            for inst in bb.instructions:
                si = getattr(inst, 'sync_info', None)
                waits = list(si.on_wait) if si is not None and si.on_wait else []
                if len(waits) > MAX_WAITS:
                    overflow, keep = waits[:-MAX_WAITS], waits[-MAX_WAITS:]
                    for i in range(0, len(overflow), MAX_WAITS):
                        chunk = overflow[i:i + MAX_WAITS]
                        _split_ctr[0] += 1
                        noop = mybir.InstNoOp(
                            name=f"wait-split-{_split_ctr[0]}",
                            engine=inst.engine,
                            sync_info=mybir.SyncInfo(on_wait=chunk, on_update=[]),
                            bass_nofuse=True,
                        )
                        new_insts.append(noop)
                    inst.sync_info = mybir.SyncInfo(on_wait=keep, on_update=si.on_update)
                new_insts.append(inst)
            bb.instructions = new_insts


def build_nc(split=True):
    nc = bass.Bass("TRN2", num_devices=8)

    x_d = nc.dram_tensor("x", [C, L], f32, kind="ExternalInput")
    gamma_d = nc.dram_tensor("gamma", [C], f32, kind="ExternalInput")
    beta_d = nc.dram_tensor("beta", [C], f32, kind="ExternalInput")
    wqkvT_d = nc.dram_tensor("wqkvT8", [C, 3 * C], fp8, kind="ExternalInput")
    woutT_d = nc.dram_tensor("woutT8", [C, C], fp8, kind="ExternalInput")
    bqk_d = nc.dram_tensor("bqk", [2 * C], f32, kind="ExternalInput")
    bvbc_d = nc.dram_tensor("bvbc", [128, C], f32, kind="ExternalInput")
    bout_d = nc.dram_tensor("bout", [C], f32, kind="ExternalInput")
    out_d = nc.dram_tensor("out", [C, L], f32, kind="ExternalOutput")

    # inline constants for group-stat aggregation
    m_c2g = np.zeros((C, G), np.float32)
    for c in range(C):
        m_c2g[c, c // GS] = 1.0 / GS
    mask_c2g_d = nc.inline_tensor(m_c2g, "mask_c2g")
    m_g2c = np.zeros((G, C), np.float32)
    for c in range(C):
        m_g2c[c // GS, c] = 1.0
    mask_g2c_d = nc.inline_tensor(m_g2c, "mask_g2c")
    ones128f_d = nc.inline_tensor(np.full((128, 128), 0.125, np.float32), "ones128f")

    with tile.TileContext(nc) as tc:
        with tc.tile_pool(name="singles", bufs=1) as singles:
            # ---- weight/bias/const loads (gpsimd queue: off the x-stream path) ----
            # wq[p]: [128, 2, 3C] fp8, [:, j, :] = c-tile 2p+j rows of wqkvT*8
            wq = [singles.tile([128, 2, 3 * C], fp8, tag=f"wq{p}", name=f"wq{p}")
                  for p in range(NP)]
            for p in range(NP):
                for j in range(2):
                    t = 2 * p + j
                    nc.gpsimd.dma_start(out=wq[p][:, j, :],
                                        in_=wqkvT_d[t * 128:(t + 1) * 128, :])
            wo = [singles.tile([128, 2, C], fp8, tag=f"wo{p}", name=f"wo{p}")
                  for p in range(NP)]
            for p in range(NP):
                for j in range(2):
                    t = 2 * p + j
                    nc.gpsimd.dma_start(out=wo[p][:, j, :],
                                        in_=woutT_d[t * 128:(t + 1) * 128, :])
            bqk_sb = singles.tile([128, 2 * CT], f32, tag="bqk", name="bqk")
            nc.gpsimd.dma_start(out=bqk_sb, in_=bqk_d[:].rearrange("(t p) -> p t", p=128))
            bvbc = singles.tile([128, C], f32, tag="bvbc", name="bvbc")
            nc.gpsimd.dma_start(out=bvbc, in_=bvbc_d[:, :])
            bout_sb = singles.tile([128, CT], f32, tag="bout", name="bout")
            nc.gpsimd.dma_start(out=bout_sb, in_=bout_d[:].rearrange("(t p) -> p t", p=128))
            gamma_sb = singles.tile([128, CT], f32, tag="gamma", name="gamma")
            nc.gpsimd.dma_start(out=gamma_sb, in_=gamma_d[:].rearrange("(t p) -> p t", p=128))
            beta_sb = singles.tile([128, CT], f32, tag="beta", name="beta")
            nc.gpsimd.dma_start(out=beta_sb, in_=beta_d[:].rearrange("(t p) -> p t", p=128))
            mask_c2g = [singles.tile([128, G], f32, tag=f"mc2g{t}", name=f"mc2g{t}")
                        for t in range(CT)]
            for t in range(CT):
                nc.gpsimd.dma_start(out=mask_c2g[t], in_=mask_c2g_d[t * 128:(t + 1) * 128, :])
            mask_g2c = singles.tile([G, C], f32, tag="mg2c", name="mg2c")
            nc.gpsimd.dma_start(out=mask_g2c, in_=mask_g2c_d[:, :])
            # 0.125-valued f32r ones for the softmax denominator matmul
            # (0.125 pre-folds the ao*64 / w*8 rescale into den)
            ones128f = singles.tile([128, 128], f32r, tag="ones128f", name="ones128f")
            nc.gpsimd.dma_start(out=ones128f, in_=ones128f_d[:, :].bitcast(f32r))
            eps8 = singles.tile([G, 1], f32, tag="eps8", name="eps8")
            nc.vector.memset(eps8, EPS)
            negc0 = singles.tile([128, 1], f32, tag="negc0", name="negc0")
            nc.vector.memset(negc0, -C0)

            # per-channel norm scale/offset (computed in stats phase)
            s_c = [singles.tile([128, 1], f32, tag=f"s_c{t}", name=f"s_c{t}") for t in range(CT)]
            t_c = [singles.tile([128, 1], f32, tag=f"t_c{t}", name=f"t_c{t}") for t in range(CT)]

            # fp8 pair tensors for attention:
            #   q/k[p]: [128(c), 2(c-pair), L];  vp[g]: [128(l), 2(kt-pair), C]
            q_sb = [singles.tile([128, 2, L], fp8, tag=f"q{p}", name=f"q{p}") for p in range(NP)]
            k_sb = [singles.tile([128, 2, L], fp8, tag=f"k{p}", name=f"k{p}") for p in range(NP)]
            vp = [singles.tile([128, 2, C], fp8, tag=f"vp{g}", name=f"vp{g}") for g in range(KP)]

            # f32 x stays resident for the whole kernel (stats, xn, residual)
            x_sb = [singles.tile([128, L], f32, tag=f"x{t}", name=f"x{t}") for t in range(CT)]

            # ---- Phase A: GroupNorm statistics (stream x once, keep it) ----
            with (
                tc.tile_pool(name="stats", bufs=1) as stats,
                tc.tile_pool(name="statps", bufs=1, space="PSUM") as statps,
                tc.tile_pool(name="stmp", bufs=8) as stmp,
            ):
                bn = [stats.tile([128, LC, 6], f32, tag=f"bn{t}", name=f"bn{t}") for t in range(CT)]
                for t in range(CT):
                    for lc in range(LC):
                        xc = x_sb[t][:, lc * NCH:(lc + 1) * NCH]
                        eng = (nc.sync, nc.gpsimd, nc.scalar)[(t * LC + lc) % 3]
                        eng.dma_start(
                            out=xc, in_=x_d[t * 128:(t + 1) * 128, lc * NCH:(lc + 1) * NCH])
                        nc.vector.bn_stats(out=bn[t][:, lc, :], in_=xc)
                # per-channel mean/E[x^2] -> stats2[t] [128,2]
                stats2 = [stats.tile([128, 2], f32, tag=f"st2{t}", name=f"st2{t}") for t in range(CT)]
                for t in range(CT):
                    mv = stmp.tile([128, 2], f32, tag="mv", name="mv")
                    nc.vector.bn_aggr(out=mv, in_=bn[t])
                    sq = stmp.tile([128, 1], f32, tag="sq", name="sq")
                    nc.vector.tensor_mul(out=sq, in0=mv[:, 0:1], in1=mv[:, 0:1])
                    nc.vector.tensor_copy(out=stats2[t][:, 0:1], in_=mv[:, 0:1])
                    nc.vector.tensor_add(out=stats2[t][:, 1:2], in0=mv[:, 1:2], in1=sq)
                # group aggregation: [8,2] = sum_t mask_c2g[t]^T @ stats2[t]
                ps_g = statps.tile([G, 2], f32, tag="psg", name="psg")
                for t in range(CT):
                    nc.tensor.matmul(ps_g, lhsT=mask_c2g[t], rhs=stats2[t],
                                     start=(t == 0), stop=(t == CT - 1))
                gs = stmp.tile([G, 2], f32, tag="gs", name="gs")       # mean_g, E[x2]_g
                nc.vector.tensor_copy(out=gs, in_=ps_g)
                mg2 = stmp.tile([G, 1], f32, tag="mg2", name="mg2")
                varg = stmp.tile([G, 1], f32, tag="varg", name="varg")
                nc.vector.tensor_mul(out=mg2, in0=gs[:, 0:1], in1=gs[:, 0:1])
                nc.vector.tensor_sub(out=varg, in0=gs[:, 1:2], in1=mg2)
                # rstd_g = 1/sqrt(var+eps)
                nc.scalar.activation(out=varg, in_=varg,
                                     func=mybir.ActivationFunctionType.Sqrt,
                                     bias=eps8, scale=1.0)
                gstats = stmp.tile([G, 2], f32, tag="gstats", name="gstats")  # mean_g, rstd_g
                nc.vector.tensor_copy(out=gstats[:, 0:1], in_=gs[:, 0:1])
                nc.vector.reciprocal(out=gstats[:, 1:2], in_=varg)
                # broadcast to channels; s_c = gamma*rstd, t_c = beta - mean*s_c
                for t in range(CT):
                    ps_bc = statps.tile([128, 2], f32, tag="psbc", name="psbc")
                    nc.tensor.matmul(ps_bc, lhsT=mask_g2c[:, t * 128:(t + 1) * 128],
                                     rhs=gstats, start=True, stop=True)
                    bc = stmp.tile([128, 2], f32, tag="bc", name="bc")
                    nc.vector.tensor_copy(out=bc, in_=ps_bc)
                    nc.vector.tensor_mul(out=s_c[t], in0=gamma_sb[:, t:t + 1], in1=bc[:, 1:2])
                    tmp = stmp.tile([128, 1], f32, tag="tmp", name="tmp")
                    nc.vector.tensor_mul(out=tmp, in0=bc[:, 0:1], in1=s_c[t])
                    nc.vector.tensor_sub(out=t_c[t], in0=beta_sb[:, t:t + 1], in1=tmp)

            # ---- Phase B: qkv projection in fp8 DoubleRow; V^T produced directly ----
            with (
                tc.tile_pool(name="xq", bufs=6) as xq,
                tc.tile_pool(name="qps", bufs=4, space="PSUM") as qps,
                tc.tile_pool(name="vps", bufs=3, space="PSUM") as vps,
            ):
                for lc in range(LC):
                    lo = lc * NCH
                    # normalized x in fp8 pairs: xn[p][:, j, :] = s_c*x + t_c for c-tile 2p+j
                    xn = []
                    for p in range(NP):
                        xn_p = xq.tile([128, 2, NCH], fp8, tag=f"xn{p}", name=f"xn{p}")
                        for j in range(2):
                            t = 2 * p + j
                            nc.vector.tensor_scalar(
                                out=xn_p[:, j, :], in0=x_sb[t][:, lo:lo + NCH],
                                scalar1=s_c[t], scalar2=t_c[t],
                                op0=mybir.AluOpType.mult,
                                op1=mybir.AluOpType.add)
                        xn.append(xn_p)
                    # Q,K rows: out [128(o), 512(l)] accumulated over both c-pairs
                    for ot in range(2 * CT):
                        ps = qps.tile([128, NCH], f32, tag="qps", name="qps")
                        # one accumulation group per PSUM bank: start only on the
                        # first matmul touching the bank (start zeroes the whole 2KB)
                        for p in range(NP):
                            for h in range(2):
                                nc.tensor.matmul(
                                    ps[:, h * 256:(h + 1) * 256],
                                    lhsT=wq[p][:, :, ot * 128:(ot + 1) * 128],
                                    rhs=xn[p][:, :, h * 256:(h + 1) * 256],
                                    start=(p == 0 and h == 0),
                                    stop=(p == NP - 1 and h == 1),
                                    skip_group_check=True,
                                    perf_mode=DR)
                        # (ps*0.125 + b) -> fp8 q/k pair slot  (gpsimd can't read PSUM)
                        dst = q_sb if ot < CT else k_sb
                        tt = ot if ot < CT else ot - CT
                        nc.scalar.activation(
                            out=dst[tt // 2][:, tt % 2, lo:lo + NCH], in_=ps,
                            func=mybir.ActivationFunctionType.Identity,
                            bias=bqk_sb[:, ot:ot + 1], scale=1.0 / WS)
                    # V^T: out [128(l), C] = xn^T @ wvT, per 128-l block
                    for lt in range(CT):
                        psv = vps.tile([128, C], f32, tag="vps", name="vps")
                        for p in range(NP):
                            for h in range(2):
                                nc.tensor.matmul(
                                    psv[:, h * 256:(h + 1) * 256],
                                    lhsT=xn[p][:, :, lt * 128:(lt + 1) * 128],
                                    rhs=wq[p][:, :, 2 * C + h * 256:2 * C + (h + 1) * 256],
                                    start=(p == 0 and h == 0),
                                    stop=(p == NP - 1 and h == 1),
                                    skip_group_check=True,
                                    perf_mode=DR)
                        g = lc * 2 + lt // 2
                        nc.vector.scalar_tensor_tensor(
                            out=vp[g][:, lt % 2, :], in0=psv,
                            scalar=1.0 / WS, in1=bvbc,
                            op0=mybir.AluOpType.mult,
                            op1=mybir.AluOpType.add)

            # ---- Phase C: attention + output projection + residual ----
            with (
                tc.tile_pool(name="exps", bufs=1) as exps,
                tc.tile_pool(name="sps", bufs=3, space="PSUM") as sps,
                tc.tile_pool(name="ops", bufs=1, space="PSUM") as ops,
                tc.tile_pool(name="dps", bufs=1, space="PSUM") as dps,
                tc.tile_pool(name="cwork", bufs=2) as cwork,
                tc.tile_pool(name="yout", bufs=4) as yout,
            ):
                for lc in range(LC):
                    lo = lc * NCH
                    ps_o = [ops.tile([128, NCH], f32, tag=f"o{ct}", name=f"o{ct}") for ct in range(CT)]
                    # den/8 accumulated via f32r ones(0.125) on DVE/gpsimd partial sums
                    ps_den = dps.tile([128, NCH], f32, tag="den", name="den")
                    for g in range(KP):
                        ep = exps.tile([128, 2, NCH], fp8, tag=f"e{g}", name=f"e{g}")
                        for j in range(2):
                            kt = 2 * g + j
                            ps_s = sps.tile([128, NCH], f32, tag="s", name="s")
                            for p in range(NP):
                                for h in range(2):
                                    nc.tensor.matmul(
                                        ps_s[:, h * 256:(h + 1) * 256],
                                        lhsT=k_sb[p][:, :, kt * 128:(kt + 1) * 128],
                                        rhs=q_sb[p][:, :, lo + h * 256:lo + (h + 1) * 256],
                                        start=(p == 0 and h == 0),
                                        stop=(p == NP - 1 and h == 1),
                                        skip_group_check=True,
                                        perf_mode=DR)
                            # E = exp(s*SCALE - C0) in fp8 (offset cancels in softmax)
                            nc.scalar.activation(out=ep[:, j, :], in_=ps_s,
                                                 func=mybir.ActivationFunctionType.Exp,
                                                 bias=negc0, scale=SCALE)
                        # pair-sum E (gpsimd, SBUF only) -> half of a quad-sum tile
                        if g % 2 == 0:
                            qs = cwork.tile([128, 2, NCH], f32, tag="qs", name="qs")
                        nc.gpsimd.tensor_add(out=qs[:, g % 2, :],
                                             in0=ep[:, 0, :], in1=ep[:, 1, :])
                        if g % 2 == 1:
                            u = cwork.tile([128, NCH], f32r, tag="u", name="u")
                            nc.vector.tensor_add(out=u, in0=qs[:, 0, :],
                                                 in1=qs[:, 1, :])
                            nc.tensor.matmul(ps_den, lhsT=ones128f, rhs=u,
                                             start=(g == 1), stop=(g == KP - 1))
                        for ct in range(CT):
                            for h in range(2):
                                nc.tensor.matmul(
                                    ps_o[ct][:, h * 256:(h + 1) * 256],
                                    lhsT=vp[g][:, :, ct * 128:(ct + 1) * 128],
                                    rhs=ep[:, :, h * 256:(h + 1) * 256],
                                    start=(g == 0 and h == 0),
                                    stop=(g == KP - 1 and h == 1),
                                    skip_group_check=True,
                                    perf_mode=DR)
                    # den_r = 8/den (ones were 0.125); with w*8 and ao/64 this yields W@ao/den
                    den_r = cwork.tile([128, NCH], f32, tag="den_r", name="den_r")
                    nc.vector.reciprocal(out=den_r, in_=ps_den)
                    ao = []
                    for p in range(NP):
                        a = cwork.tile([128, 2, NCH], fp8, tag=f"ao{p}", name=f"ao{p}")
                        for j in range(2):
                            nc.vector.tensor_scalar(
                                out=a[:, j, :], in0=ps_o[2 * p + j],
                                scalar1=AOS, scalar2=None,
                                op0=mybir.AluOpType.mult)
                        ao.append(a)
                    for ot in range(CT):
                        ps_p = sps.tile([128, NCH], f32, tag="s", name="s")
                        for p in range(NP):
                            for h in range(2):
                                nc.tensor.matmul(
                                    ps_p[:, h * 256:(h + 1) * 256],
                                    lhsT=wo[p][:, :, ot * 128:(ot + 1) * 128],
                                    rhs=ao[p][:, :, h * 256:(h + 1) * 256],
                                    start=(p == 0 and h == 0),
                                    stop=(p == NP - 1 and h == 1),
                                    skip_group_check=True,
                                    perf_mode=DR)
                        y1 = yout.tile([128, NCH], f32, tag="y1", name="y1")
                        nc.vector.tensor_mul(out=y1, in0=ps_p, in1=den_r)
                        y = yout.tile([128, NCH], f32, tag="y", name="y")
                        nc.vector.scalar_tensor_tensor(
                            out=y, in0=y1, scalar=bout_sb[:, ot:ot + 1],
                            in1=x_sb[ot][:, lo:lo + NCH],
                            op0=mybir.AluOpType.add,
                            op1=mybir.AluOpType.add)
                        nc.sync.dma_start(
                            out=out_d[ot * 128:(ot + 1) * 128, lo:lo + NCH], in_=y)

    if split:
        _split_multi_waits(nc)
    return nc


_NC_CACHE = [None]


def make_in_maps(x, gamma, beta, w_qkv, b_qkv, w_out, b_out):
    x = np.ascontiguousarray(np.asarray(x, dtype=np.float32))
    w_qkv = np.asarray(w_qkv, np.float32)
    b_qkv = np.asarray(b_qkv, np.float32)
    common = {
        "gamma": np.ascontiguousarray(np.asarray(gamma, np.float32)),
        "beta": np.ascontiguousarray(np.asarray(beta, np.float32)),
        "wqkvT8": np.ascontiguousarray((w_qkv.T * WS).astype(npfp8)),
        "bqk": np.ascontiguousarray(b_qkv[:2 * C]),
        "bvbc": np.ascontiguousarray(np.broadcast_to(b_qkv[2 * C:], (128, C)).copy()),
        "woutT8": np.ascontiguousarray((np.asarray(w_out, np.float32).T * WS).astype(npfp8)),
        "bout": np.ascontiguousarray(np.asarray(b_out, np.float32)),
    }
    return [dict(common, x=np.ascontiguousarray(x[i])) for i in range(B)]


def kernel(x, gamma, beta, w_qkv, b_qkv, w_out, b_out):
    if _NC_CACHE[0] is None:
        _NC_CACHE[0] = build_nc()
    in_maps = make_in_maps(x, gamma, beta, w_qkv, b_qkv, w_out, b_out)
    res = run_bass_kernel_spmd(_NC_CACHE[0], in_maps, core_ids=list(range(B)))
    out = np.stack([res.results[i]["out"] for i in range(B)], axis=0)
    return out.astype(np.float32)


# revision 21
# speedup vs baseline: 1.8833x; 1.1695x over previous
"""AttentionBlock (GroupNorm -> qkv -> single-head attention L=4096 -> proj -> residual)
on 8 Trainium2 NeuronCores, data-parallel over the batch (B=8, one batch element per core).

v2: all four GEMMs in fp8e4 DoubleRow (2x PE throughput).
 - q/k/v/E/attn-out quantized to fp8 with static scales (w*8, ao/64, exp offset e^-3),
   all of which cancel exactly in the softmax ratio / final rescale.
 - V^T produced directly from the qkv matmul by swapping stationary/moving operands
   (no PE transposes, no transpose copies).
 - softmax denominator via 0.125-valued fp8 ones matmul on E pairs (no DVE add tree);
   the 0.125 pre-folds the ao*64 rescale: recip(den/8)*W8*(ao/64) = W*ao/den.
 - scalar engine runs exp only; bias/convert ops on gpsimd; residual+bias fused into
   one DVE scalar_tensor_tensor reading the f32 x kept resident in SBUF.

Self-contained: hardcodes shapes B=8, C=512, L=4096, GROUPS=8.
"""
import sys
sys.path.insert(0, '/opt/trn_rl_repo')
import numpy as np
import concourse.bass as bass
import concourse.tile as tile
from concourse import mybir
from concourse.bass_utils import run_bass_kernel_spmd

B, C, L = 8, 512, 4096
G = 8                    # groups
GS = C // G              # 64 channels per group
CT = C // 128            # 4 channel partition-tiles
NP = CT // 2             # 2 channel pair-tiles (DoubleRow contraction pairs)
NCH = 512                # column chunk width
LC = L // NCH            # 8 l-chunks
KT = L // 128            # 32 k partition tiles
KP = KT // 2             # 16 k pair-tiles
EPS = 1e-5
SCALE = 1.0 / float(np.sqrt(C))
C0 = 3.0                 # global exp offset: exp(s*SCALE - C0); cancels in softmax
WS = 8.0                 # weight quantization scale (w*8 in fp8)
AOS = 1.0 / 64.0         # attention-out quantization scale

f32 = mybir.dt.float32
f32r = mybir.dt.float32r
bf16 = mybir.dt.bfloat16
fp8 = mybir.dt.float8e4
npfp8 = mybir.dt.np(fp8)
DR = mybir.MatmulPerfMode.DoubleRow

MAX_WAITS = 1
_split_ctr = [0]


def _split_multi_waits(nc):
    """walrus in this container rejects >1 sync wait per instruction.
    Hoist overflow waits onto same-engine NoOps inserted just before."""
    for f in nc.m.functions:
        for bb in f.blocks:
            new_insts = []
            for inst in bb.instructions:
                si = getattr(inst, 'sync_info', None)
                waits = list(si.on_wait) if si is not None and si.on_wait else []
                if len(waits) > MAX_WAITS:
                    overflow, keep = waits[:-MAX_WAITS], waits[-MAX_WAITS:]
                    for i in range(0, len(overflow), MAX_WAITS):
                        chunk = overflow[i:i + MAX_WAITS]
                        _split_ctr[0] += 1
                        noop = mybir.InstNoOp(
                            name=f"wait-split-{_split_ctr[0]}",
                            engine=inst.engine,
                            sync_info=mybir.SyncInfo(on_wait=chunk, on_update=[]),
                            bass_nofuse=True,
                        )
                        new_insts.append(noop)
                    inst.sync_info = mybir.SyncInfo(on_wait=keep, on_update=si.on_update)
                new_insts.append(inst)
            bb.instructions = new_insts


def build_nc(split=True):
    nc = bass.Bass("TRN2", num_devices=8)

    x_d = nc.dram_tensor("xh", [C, L], bf16, kind="ExternalInput")
    gamma_d = nc.dram_tensor("gamma", [C], f32, kind="ExternalInput")
    beta_d = nc.dram_tensor("beta", [C], f32, kind="ExternalInput")
    wqkvT_d = nc.dram_tensor("wqkvT8", [C, 3 * C], fp8, kind="ExternalInput")
    woutT_d = nc.dram_tensor("woutT8", [C, C], fp8, kind="ExternalInput")
    bqk_d = nc.dram_tensor("bqk", [2 * C], f32, kind="ExternalInput")
    bvbc_d = nc.dram_tensor("bvbc", [128, C], f32, kind="ExternalInput")
    bout_d = nc.dram_tensor("bout", [C], f32, kind="ExternalInput")
    out_d = nc.dram_tensor("out", [C, L], f32, kind="ExternalOutput")

    # inline constants for group-stat aggregation
    m_c2g = np.zeros((C, G), np.float32)
    for c in range(C):
        m_c2g[c, c // GS] = 1.0 / GS
    mask_c2g_d = nc.inline_tensor(m_c2g, "mask_c2g")
    m_g2c = np.zeros((G, C), np.float32)
    for c in range(C):
        m_g2c[c // GS, c] = 1.0
    mask_g2c_d = nc.inline_tensor(m_g2c, "mask_g2c")
    ones128f_d = nc.inline_tensor(np.full((128, 128), 0.125, np.float32), "ones128f")

    with tile.TileContext(nc) as tc:
        with tc.tile_pool(name="singles", bufs=1) as singles:
            # ---- weight/bias/const loads (gpsimd queue: off the x-stream path) ----
            # wq[p]: [128, 2, 3C] fp8, [:, j, :] = c-tile 2p+j rows of wqkvT*8
            wq = [singles.tile([128, 2, 3 * C], fp8, tag=f"wq{p}", name=f"wq{p}")
                  for p in range(NP)]
            for p in range(NP):
                for j in range(2):
                    t = 2 * p + j
                    nc.gpsimd.dma_start(out=wq[p][:, j, :],
                                        in_=wqkvT_d[t * 128:(t + 1) * 128, :])
            wo = [singles.tile([128, 2, C], fp8, tag=f"wo{p}", name=f"wo{p}")
                  for p in range(NP)]
            for p in range(NP):
                for j in range(2):
                    t = 2 * p + j
                    nc.gpsimd.dma_start(out=wo[p][:, j, :],
                                        in_=woutT_d[t * 128:(t + 1) * 128, :])
            bqk_sb = singles.tile([128, 2 * CT], f32, tag="bqk", name="bqk")
            nc.gpsimd.dma_start(out=bqk_sb, in_=bqk_d[:].rearrange("(t p) -> p t", p=128))
            bvbc = singles.tile([128, C], f32, tag="bvbc", name="bvbc")
            nc.gpsimd.dma_start(out=bvbc, in_=bvbc_d[:, :])
            bout_sb = singles.tile([128, CT], f32, tag="bout", name="bout")
            nc.gpsimd.dma_start(out=bout_sb, in_=bout_d[:].rearrange("(t p) -> p t", p=128))
            gamma_sb = singles.tile([128, CT], f32, tag="gamma", name="gamma")
            nc.gpsimd.dma_start(out=gamma_sb, in_=gamma_d[:].rearrange("(t p) -> p t", p=128))
            beta_sb = singles.tile([128, CT], f32, tag="beta", name="beta")
            nc.gpsimd.dma_start(out=beta_sb, in_=beta_d[:].rearrange("(t p) -> p t", p=128))
            mask_c2g = [singles.tile([128, G], f32, tag=f"mc2g{t}", name=f"mc2g{t}")
                        for t in range(CT)]
            for t in range(CT):
                nc.gpsimd.dma_start(out=mask_c2g[t], in_=mask_c2g_d[t * 128:(t + 1) * 128, :])
            mask_g2c = singles.tile([G, C], f32, tag="mg2c", name="mg2c")
            nc.gpsimd.dma_start(out=mask_g2c, in_=mask_g2c_d[:, :])
            # 0.125-valued f32r ones for the softmax denominator matmul
            # (0.125 pre-folds the ao*64 / w*8 rescale into den)
            ones128f = singles.tile([128, 128], f32r, tag="ones128f", name="ones128f")
            nc.gpsimd.dma_start(out=ones128f, in_=ones128f_d[:, :].bitcast(f32r))
            eps8 = singles.tile([G, 1], f32, tag="eps8", name="eps8")
            nc.vector.memset(eps8, EPS)
            negc0 = singles.tile([128, 1], f32, tag="negc0", name="negc0")
            nc.vector.memset(negc0, -C0)

            # per-channel norm scale/offset (computed in stats phase)
            s_c = [singles.tile([128, 1], f32, tag=f"s_c{t}", name=f"s_c{t}") for t in range(CT)]
            t_c = [singles.tile([128, 1], f32, tag=f"t_c{t}", name=f"t_c{t}") for t in range(CT)]

            # fp8 pair tensors for attention:
            #   q/k[p]: [128(c), 2(c-pair), L];  vp[g]: [128(l), 2(kt-pair), C]
            q_sb = [singles.tile([128, 2, L], fp8, tag=f"q{p}", name=f"q{p}") for p in range(NP)]
            k_sb = [singles.tile([128, 2, L], fp8, tag=f"k{p}", name=f"k{p}") for p in range(NP)]
            vp = [singles.tile([128, 2, C], fp8, tag=f"vp{g}", name=f"vp{g}") for g in range(KP)]

            # bf16 x stays resident for the whole kernel (stats, xn, residual)
            x_sb = [singles.tile([128, L], bf16, tag=f"x{t}", name=f"x{t}") for t in range(CT)]

            # ---- Phase A: GroupNorm statistics (stream x once, keep it) ----
            with (
                tc.tile_pool(name="stats", bufs=1) as stats,
                tc.tile_pool(name="statps", bufs=1, space="PSUM") as statps,
                tc.tile_pool(name="stmp", bufs=8) as stmp,
            ):
                bn = [stats.tile([128, LC, 6], f32, tag=f"bn{t}", name=f"bn{t}") for t in range(CT)]
                for t in range(CT):
                    for lc in range(LC):
                        xc = x_sb[t][:, lc * NCH:(lc + 1) * NCH]
                        eng = (nc.sync, nc.gpsimd, nc.scalar)[(t * LC + lc) % 3]
                        eng.dma_start(
                            out=xc, in_=x_d[t * 128:(t + 1) * 128, lc * NCH:(lc + 1) * NCH])
                        nc.vector.bn_stats(out=bn[t][:, lc, :], in_=xc)
                # per-channel mean/E[x^2] -> stats2[t] [128,2]
                stats2 = [stats.tile([128, 2], f32, tag=f"st2{t}", name=f"st2{t}") for t in range(CT)]
                for t in range(CT):
                    mv = stmp.tile([128, 2], f32, tag="mv", name="mv")
                    nc.vector.bn_aggr(out=mv, in_=bn[t])
                    sq = stmp.tile([128, 1], f32, tag="sq", name="sq")
                    nc.vector.tensor_mul(out=sq, in0=mv[:, 0:1], in1=mv[:, 0:1])
                    nc.vector.tensor_copy(out=stats2[t][:, 0:1], in_=mv[:, 0:1])
                    nc.vector.tensor_add(out=stats2[t][:, 1:2], in0=mv[:, 1:2], in1=sq)
                # group aggregation: [8,2] = sum_t mask_c2g[t]^T @ stats2[t]
                ps_g = statps.tile([G, 2], f32, tag="psg", name="psg")
                for t in range(CT):
                    nc.tensor.matmul(ps_g, lhsT=mask_c2g[t], rhs=stats2[t],
                                     start=(t == 0), stop=(t == CT - 1))
                gs = stmp.tile([G, 2], f32, tag="gs", name="gs")       # mean_g, E[x2]_g
                nc.vector.tensor_copy(out=gs, in_=ps_g)
                mg2 = stmp.tile([G, 1], f32, tag="mg2", name="mg2")
                varg = stmp.tile([G, 1], f32, tag="varg", name="varg")
                nc.vector.tensor_mul(out=mg2, in0=gs[:, 0:1], in1=gs[:, 0:1])
                nc.vector.tensor_sub(out=varg, in0=gs[:, 1:2], in1=mg2)
                # rstd_g = 1/sqrt(var+eps)
                nc.scalar.activation(out=varg, in_=varg,
                                     func=mybir.ActivationFunctionType.Sqrt,
                                     bias=eps8, scale=1.0)
                gstats = stmp.tile([G, 2], f32, tag="gstats", name="gstats")  # mean_g, rstd_g
                nc.vector.tensor_copy(out=gstats[:, 0:1], in_=gs[:, 0:1])
                nc.vector.reciprocal(out=gstats[:, 1:2], in_=varg)
                # broadcast to channels; s_c = gamma*rstd, t_c = beta - mean*s_c
                for t in range(CT):
                    ps_bc = statps.tile([128, 2], f32, tag="psbc", name="psbc")
                    nc.tensor.matmul(ps_bc, lhsT=mask_g2c[:, t * 128:(t + 1) * 128],
                                     rhs=gstats, start=True, stop=True)
                    bc = stmp.tile([128, 2], f32, tag="bc", name="bc")
                    nc.vector.tensor_copy(out=bc, in_=ps_bc)
                    nc.vector.tensor_mul(out=s_c[t], in0=gamma_sb[:, t:t + 1], in1=bc[:, 1:2])
                    tmp = stmp.tile([128, 1], f32, tag="tmp", name="tmp")
                    nc.vector.tensor_mul(out=tmp, in0=bc[:, 0:1], in1=s_c[t])
                    nc.vector.tensor_sub(out=t_c[t], in0=beta_sb[:, t:t + 1], in1=tmp)

            # ---- Phase B: qkv projection in fp8 DoubleRow; V^T produced directly ----
            with (
                tc.tile_pool(name="xq", bufs=6) as xq,
                tc.tile_pool(name="qps", bufs=4, space="PSUM") as qps,
                tc.tile_pool(name="vps", bufs=3, space="PSUM") as vps,
            ):
                for lc in range(LC):
                    lo = lc * NCH
                    # normalized x in fp8 pairs: xn[p][:, j, :] = s_c*x + t_c for c-tile 2p+j
                    xn = []
                    for p in range(NP):
                        xn_p = xq.tile([128, 2, NCH], fp8, tag=f"xn{p}", name=f"xn{p}")
                        for j in range(2):
                            t = 2 * p + j
                            nc.vector.tensor_scalar(
                                out=xn_p[:, j, :], in0=x_sb[t][:, lo:lo + NCH],
                                scalar1=s_c[t], scalar2=t_c[t],
                                op0=mybir.AluOpType.mult,
                                op1=mybir.AluOpType.add)
                        xn.append(xn_p)
                    # Q,K rows: out [128(o), 512(l)] accumulated over both c-pairs
                    for ot in range(2 * CT):
                        ps = qps.tile([128, NCH], f32, tag="qps", name="qps")
                        # one accumulation group per PSUM bank: start only on the
                        # first matmul touching the bank (start zeroes the whole 2KB)
                        for p in range(NP):
                            for h in range(2):
                                nc.tensor.matmul(
                                    ps[:, h * 256:(h + 1) * 256],
                                    lhsT=wq[p][:, :, ot * 128:(ot + 1) * 128],
                                    rhs=xn[p][:, :, h * 256:(h + 1) * 256],
                                    start=(p == 0 and h == 0),
                                    stop=(p == NP - 1 and h == 1),
                                    skip_group_check=True,
                                    perf_mode=DR)
                        # (ps*0.125 + b) -> fp8 q/k pair slot  (gpsimd can't read PSUM)
                        dst = q_sb if ot < CT else k_sb
                        tt = ot if ot < CT else ot - CT
                        nc.scalar.activation(
                            out=dst[tt // 2][:, tt % 2, lo:lo + NCH], in_=ps,
                            func=mybir.ActivationFunctionType.Identity,
                            bias=bqk_sb[:, ot:ot + 1], scale=1.0 / WS)
                    # V^T: out [128(l), C] = xn^T @ wvT, per 128-l block
                    for lt in range(CT):
                        psv = vps.tile([128, C], f32, tag="vps", name="vps")
                        for p in range(NP):
                            for h in range(2):
                                nc.tensor.matmul(
                                    psv[:, h * 256:(h + 1) * 256],
                                    lhsT=xn[p][:, :, lt * 128:(lt + 1) * 128],
                                    rhs=wq[p][:, :, 2 * C + h * 256:2 * C + (h + 1) * 256],
                                    start=(p == 0 and h == 0),
                                    stop=(p == NP - 1 and h == 1),
                                    skip_group_check=True,
                                    perf_mode=DR)
                        g = lc * 2 + lt // 2
                        nc.vector.scalar_tensor_tensor(
                            out=vp[g][:, lt % 2, :], in0=psv,
                            scalar=1.0 / WS, in1=bvbc,
                            op0=mybir.AluOpType.mult,
                            op1=mybir.AluOpType.add)

            # ---- Phase C: attention + output projection + residual ----
            with (
                tc.tile_pool(name="exps", bufs=1) as exps,
                tc.tile_pool(name="sps", bufs=3, space="PSUM") as sps,
                tc.tile_pool(name="ops", bufs=1, space="PSUM") as ops,
                tc.tile_pool(name="dps", bufs=1, space="PSUM") as dps,
                tc.tile_pool(name="cwork", bufs=2) as cwork,
                tc.tile_pool(name="yout", bufs=4) as yout,
            ):
                def make_proj(lc, ot, ao, den_r):
                    # emits the out-projection + residual for (lc, ot); deferred into
                    # the next lc's g-loop so Tensor never waits on the ao converts
                    def emit():
                        lo = lc * NCH
                        ps_p = sps.tile([128, NCH], f32, tag="s", name="s")
                        for p in range(NP):
                            for h in range(2):
                                nc.tensor.matmul(
                                    ps_p[:, h * 256:(h + 1) * 256],
                                    lhsT=wo[p][:, :, ot * 128:(ot + 1) * 128],
                                    rhs=ao[p][:, :, h * 256:(h + 1) * 256],
                                    start=(p == 0 and h == 0),
                                    stop=(p == NP - 1 and h == 1),
                                    skip_group_check=True,
                                    perf_mode=DR)
                        y1 = yout.tile([128, NCH], f32, tag="y1", name="y1")
                        nc.vector.tensor_mul(out=y1, in0=ps_p, in1=den_r)
                        y = yout.tile([128, NCH], f32, tag="y", name="y")
                        nc.vector.scalar_tensor_tensor(
                            out=y, in0=y1, scalar=bout_sb[:, ot:ot + 1],
                            in1=x_sb[ot][:, lo:lo + NCH],
                            op0=mybir.AluOpType.add,
                            op1=mybir.AluOpType.add)
                        nc.sync.dma_start(
                            out=out_d[ot * 128:(ot + 1) * 128, lo:lo + NCH], in_=y)
                    return emit

                pend = []
                for lc in range(LC):
                    lo = lc * NCH
                    ps_o = [ops.tile([128, NCH], f32, tag=f"o{ct}", name=f"o{ct}") for ct in range(CT)]
                    # den/8 accumulated via f32r ones(0.125) on DVE/gpsimd partial sums
                    ps_den = dps.tile([128, NCH], f32, tag="den", name="den")
                    for g in range(KP):
                        ep = exps.tile([128, 2, NCH], fp8, tag=f"e{g}", name=f"e{g}")
                        for j in range(2):
                            kt = 2 * g + j
                            ps_s = sps.tile([128, NCH], f32, tag="s", name="s")
                            for p in range(NP):
                                for h in range(2):
                                    nc.tensor.matmul(
                                        ps_s[:, h * 256:(h + 1) * 256],
                                        lhsT=k_sb[p][:, :, kt * 128:(kt + 1) * 128],
                                        rhs=q_sb[p][:, :, lo + h * 256:lo + (h + 1) * 256],
                                        start=(p == 0 and h == 0),
                                        stop=(p == NP - 1 and h == 1),
                                        skip_group_check=True,
                                        perf_mode=DR)
                            # E = exp(s*SCALE - C0) in fp8 (offset cancels in softmax)
                            nc.scalar.activation(out=ep[:, j, :], in_=ps_s,
                                                 func=mybir.ActivationFunctionType.Exp,
                                                 bias=negc0, scale=SCALE)
                        # pair-sum E (gpsimd, SBUF only) -> half of a quad-sum tile
                        if g % 2 == 0:
                            qs = cwork.tile([128, 2, NCH], f32, tag="qs", name="qs")
                        nc.gpsimd.tensor_add(out=qs[:, g % 2, :],
                                             in0=ep[:, 0, :], in1=ep[:, 1, :])
                        if g % 2 == 1:
                            u = cwork.tile([128, NCH], f32r, tag="u", name="u")
                            nc.vector.tensor_add(out=u, in0=qs[:, 0, :],
                                                 in1=qs[:, 1, :])
                            nc.tensor.matmul(ps_den, lhsT=ones128f, rhs=u,
                                             start=(g == 1), stop=(g == KP - 1))
                        for ct in range(CT):
                            for h in range(2):
                                nc.tensor.matmul(
                                    ps_o[ct][:, h * 256:(h + 1) * 256],
                                    lhsT=vp[g][:, :, ct * 128:(ct + 1) * 128],
                                    rhs=ep[:, :, h * 256:(h + 1) * 256],
                                    start=(g == 0 and h == 0),
                                    stop=(g == KP - 1 and h == 1),
                                    skip_group_check=True,
                                    perf_mode=DR)
                        if pend and g in (3, 5, 7, 9):
                            pend.pop(0)()
                    # endgame converts for this lc (scalar/vector split); proj deferred
                    ao = []
                    for p in range(NP):
                        a = cwork.tile([128, 2, NCH], fp8, tag=f"ao{p}", name=f"ao{p}")
                        for j in range(2):
                            if p == 0:
                                nc.scalar.activation(
                                    out=a[:, j, :], in_=ps_o[2 * p + j],
                                    func=mybir.ActivationFunctionType.Copy,
                                    scale=AOS)
                            else:
                                nc.vector.tensor_scalar(
                                    out=a[:, j, :], in0=ps_o[2 * p + j],
                                    scalar1=AOS, scalar2=None,
                                    op0=mybir.AluOpType.mult)
                        ao.append(a)
                    # den_r = 8/den (ones were 0.125); with w*8 and ao/64: W@ao/den
                    den_r = cwork.tile([128, NCH], f32, tag="den_r", name="den_r")
                    nc.vector.reciprocal(out=den_r, in_=ps_den)
                    pend = [make_proj(lc, ot, ao, den_r) for ot in range(CT)]
                for f in pend:
                    f()

    if split:
        _split_multi_waits(nc)
    return nc


_NC_CACHE = [None]


def make_in_maps(x, gamma, beta, w_qkv, b_qkv, w_out, b_out):
    x = np.ascontiguousarray(np.asarray(x, dtype=np.float32))
    w_qkv = np.asarray(w_qkv, np.float32)
    b_qkv = np.asarray(b_qkv, np.float32)
    common = {
        "gamma": np.ascontiguousarray(np.asarray(gamma, np.float32)),
        "beta": np.ascontiguousarray(np.asarray(beta, np.float32)),
        "wqkvT8": np.ascontiguousarray((w_qkv.T * WS).astype(npfp8)),
        "bqk": np.ascontiguousarray(b_qkv[:2 * C]),
        "bvbc": np.ascontiguousarray(np.broadcast_to(b_qkv[2 * C:], (128, C)).copy()),
        "woutT8": np.ascontiguousarray((np.asarray(w_out, np.float32).T * WS).astype(npfp8)),
        "bout": np.ascontiguousarray(np.asarray(b_out, np.float32)),
    }
    npbf16 = mybir.dt.np(bf16)
    return [dict(common, xh=np.ascontiguousarray(x[i].astype(npbf16))) for i in range(B)]


def kernel(x, gamma, beta, w_qkv, b_qkv, w_out, b_out):
    if _NC_CACHE[0] is None:
        _NC_CACHE[0] = build_nc()
    in_maps = make_in_maps(x, gamma, beta, w_qkv, b_qkv, w_out, b_out)
    res = run_bass_kernel_spmd(_NC_CACHE[0], in_maps, core_ids=list(range(B)))
    out = np.stack([res.results[i]["out"] for i in range(B)], axis=0)
    return out.astype(np.float32)
